# revision 1
# baseline (speedup 1.0000x reference)
"""Bass/Trainium2 kernel for nn_PhysicsLoss (GNN message passing physics loss).

Strategy: shard the edge dimension (3.2M edges) across 8 NeuronCores
(400K edges each). Each core:
  1. computes per-edge weights w = sigmoid(logit) / (R + X + eps) densely,
  2. loops over 3125 columns of 128 edges: indirect-DMA gathers v[src], v[dst],
     computes current = |v_src - v_dst| * w, and indirect-DMA scatter-adds
     +current at dst / -current at src into a DRAM node accumulator,
  3. all-reduces the node accumulator (and KVL partial sums) across cores,
  4. computes mean(node_sum^2) + mean(per-column unbiased var of edge_params)
     on device.
Core 0's scalar output is returned.
"""
import numpy as np

N_NODES = 100000
N_EDGES = 3200000
NCORES = 8
P = 128
EPC = N_EDGES // NCORES          # 400000 edges per core
COLS = EPC // P                  # 3125 columns of 128 edges
ACC_ROWS = 100224                # 128 * 783 >= N_NODES, node accumulator (padded)
ACC_C = ACC_ROWS // P            # 783
EPS = 1e-6

_cache = {}


def _build():
    import concourse.bass as bass
    import concourse.bacc as bacc
    import concourse.mybir as mybir
    from concourse.tile import TileContext
    from concourse.masks import make_identity

    f32 = mybir.dt.float32
    i32 = mybir.dt.int32

    nc = bacc.Bacc("TRN2", target_bir_lowering=False, debug=False, num_devices=NCORES)

    v_d = nc.dram_tensor("v", [N_NODES, 1], f32, kind="ExternalInput")
    src_d = nc.dram_tensor("src", [P, COLS], i32, kind="ExternalInput")
    dst_d = nc.dram_tensor("dst", [P, COLS], i32, kind="ExternalInput")
    log_d = nc.dram_tensor("logits", [P, COLS], f32, kind="ExternalInput")
    par_d = nc.dram_tensor("params", [P, 2 * COLS], f32, kind="ExternalInput")
    out_d = nc.dram_tensor("out", [1, 1], f32, kind="ExternalOutput")

    # internal DRAM for the node accumulator + collective bounce buffers
    acc_d = nc.dram_tensor("acc_local", [ACC_ROWS, 1], f32)
    accr_d = nc.dram_tensor("acc_red", [ACC_ROWS, 1], f32)
    prt_d = nc.dram_tensor("prt_local", [1, 8], f32)
    prtr_d = nc.dram_tensor("prt_red", [1, 8], f32)

    acc_2d = acc_d[:, :].rearrange("(p c) o -> p (c o)", p=P)
    accr_2d = accr_d[:, :].rearrange("(p c) o -> p (c o)", p=P)

    with TileContext(nc) as tc:
        with (
            tc.tile_pool(name="big", bufs=1) as big,
            tc.tile_pool(name="sm", bufs=1) as sm,
            tc.tile_pool(name="it", bufs=4) as it,
            tc.tile_pool(name="ps", bufs=2, space="PSUM") as ps,
        ):
            # ---- load edge data ----
            srct = big.tile([P, COLS], i32, tag="srct")
            nc.sync.dma_start(out=srct[:, :], in_=src_d[:, :])
            dstt = big.tile([P, COLS], i32, tag="dstt")
            nc.sync.dma_start(out=dstt[:, :], in_=dst_d[:, :])
            logt = big.tile([P, COLS], f32, tag="logt")
            nc.sync.dma_start(out=logt[:, :], in_=log_d[:, :])
            part = big.tile([P, 2 * COLS], f32, tag="part")
            nc.sync.dma_start(out=part[:, :], in_=par_d[:, :])

            # ---- zero the accumulator ----
            zt = big.tile([P, ACC_C], f32, tag="zt")
            nc.vector.memset(zt[:, :], 0.0)
            nc.sync.dma_start(out=acc_2d, in_=zt[:, :])

            # ---- dense per-edge weight: w = sigmoid(logit) / (R+X+eps) ----
            par3 = part[:, :].rearrange("p (c two) -> p c two", two=2)
            imp = big.tile([P, COLS], f32, tag="imp")
            nc.vector.tensor_tensor(
                out=imp[:, :], in0=par3[:, :, 0], in1=par3[:, :, 1],
                op=mybir.AluOpType.add,
            )
            nc.vector.tensor_scalar_add(imp[:, :], imp[:, :], EPS)
            rec = big.tile([P, COLS], f32, tag="rec")
            nc.vector.reciprocal(rec[:, :], imp[:, :])
            sig = big.tile([P, COLS], f32, tag="sig")
            nc.scalar.activation(
                sig[:, :], logt[:, :], mybir.ActivationFunctionType.Sigmoid
            )
            wt = big.tile([P, COLS], f32, tag="wt")
            nc.vector.tensor_tensor(
                out=wt[:, :], in0=sig[:, :], in1=rec[:, :],
                op=mybir.AluOpType.mult,
            )

            # ---- KVL partials: sum and sum-of-squares of R and X columns ----
            prt = sm.tile([1, 8], f32, tag="prt")
            nc.vector.memset(prt[:, :], 0.0)
            red = sm.tile([P, 1], f32, tag="red")
            sq = big.tile([P, COLS], f32, tag="sq")
            ones = sm.tile([P, 1], f32, tag="ones")
            nc.vector.memset(ones[:, :], 1.0)
            pssc = ps.tile([1, 1], f32, tag="pssc")
            for k in range(4):  # 0: R, 1: R^2, 2: X, 3: X^2
                colap = par3[:, :, k // 2]
                if k % 2 == 0:
                    nc.vector.tensor_reduce(
                        out=red[:, :], in_=colap, axis=mybir.AxisListType.X,
                        op=mybir.AluOpType.add,
                    )
                else:
                    nc.vector.tensor_tensor(
                        out=sq[:, :], in0=colap, in1=colap, op=mybir.AluOpType.mult
                    )
                    nc.vector.tensor_reduce(
                        out=red[:, :], in_=sq[:, :], axis=mybir.AxisListType.X,
                        op=mybir.AluOpType.add,
                    )
                nc.tensor.matmul(
                    pssc[:, :], lhsT=ones[:, :], rhs=red[:, :], start=True, stop=True
                )
                nc.vector.tensor_copy(prt[:, k:k + 1], pssc[:, :])
            nc.sync.dma_start(out=prt_d[:, :], in_=prt[:, :])

            # ---- constants for dup-merge ----
            ident = sm.tile([P, P], f32, tag="ident")
            make_identity(nc, ident[:, :])
            # LT[i, j] = 1 if j < i  (strictly lower triangular)
            iota_f = sm.tile([P, P], f32, tag="iota_f")
            nc.gpsimd.iota(iota_f[:, :], pattern=[[1, P]], base=0, channel_multiplier=0, allow_small_or_imprecise_dtypes=True)
            iota_p = sm.tile([P, 1], f32, tag="iota_p")
            nc.gpsimd.iota(iota_p[:, :], pattern=[[0, 1]], base=0, channel_multiplier=1, allow_small_or_imprecise_dtypes=True)
            lt = sm.tile([P, P], f32, tag="lt")
            nc.vector.tensor_scalar(
                out=lt[:, :], in0=iota_f[:, :], scalar1=iota_p[:, :1], scalar2=None,
                op0=mybir.AluOpType.is_lt,
            )
            bigc = sm.tile([P, 1], i32, tag="bigc")
            nc.vector.memset(bigc[:, :], 1 << 28)

            # ---- main loop over 3125 columns of 128 edges ----
            def body(i):
                sc = it.tile([P, 1], i32, tag="sc")
                nc.vector.tensor_copy(sc[:, :], srct[:, bass.ds(i, 1)])
                dc = it.tile([P, 1], i32, tag="dc")
                nc.vector.tensor_copy(dc[:, :], dstt[:, bass.ds(i, 1)])
                gs = it.tile([P, 1], f32, tag="gs")
                gd = it.tile([P, 1], f32, tag="gd")
                cur = it.tile([P, 1], f32, tag="cur")
                ncur = it.tile([P, 1], f32, tag="ncur")
                idxf = it.tile([P, 1], f32, tag="idxf")
                idxT = it.tile([P, P], f32, tag="idxT")
                selm = it.tile([P, P], f32, tag="selm")
                lowm = it.tile([P, P], f32, tag="lowm")
                cnt = it.tile([P, 1], f32, tag="cnt")
                fmask = it.tile([P, 1], i32, tag="fmask")
                offs = it.tile([P, 1], i32, tag="offs")
                mrg = it.tile([P, 1], f32, tag="mrg")
                psT = ps.tile([P, P], f32, tag="psT")
                psM = ps.tile([P, 1], f32, tag="psM")
                nc.gpsimd.indirect_dma_start(
                    out=gs[:, :], out_offset=None, in_=v_d[:, :],
                    in_offset=bass.IndirectOffsetOnAxis(ap=sc[:, :], axis=0),
                )
                nc.gpsimd.indirect_dma_start(
                    out=gd[:, :], out_offset=None, in_=v_d[:, :],
                    in_offset=bass.IndirectOffsetOnAxis(ap=dc[:, :], axis=0),
                )
                nc.vector.tensor_tensor(
                    out=cur[:, :], in0=gs[:, :], in1=gd[:, :],
                    op=mybir.AluOpType.subtract,
                )
                nc.vector.tensor_scalar_mul(ncur[:, :], cur[:, :], -1.0)
                nc.vector.tensor_tensor(
                    out=cur[:, :], in0=cur[:, :], in1=ncur[:, :],
                    op=mybir.AluOpType.max,
                )
                nc.vector.tensor_tensor(
                    out=cur[:, :], in0=cur[:, :], in1=wt[:, bass.ds(i, 1)],
                    op=mybir.AluOpType.mult,
                )
                nc.vector.tensor_scalar_mul(ncur[:, :], cur[:, :], -1.0)
                for idxcol, valcol in ((dc, cur), (sc, ncur)):
                    nc.vector.tensor_copy(idxf[:, :], idxcol[:, :])
                    nc.tensor.transpose(
                        out=psT[:, :],
                        in_=idxf[:, :].to_broadcast([P, P]),
                        identity=ident[:, :],
                    )
                    nc.vector.tensor_copy(idxT[:, :], psT[:, :])
                    nc.vector.tensor_tensor(
                        out=selm[:, :], in0=idxf[:, :].to_broadcast([P, P])[:],
                        in1=idxT[:, :], op=mybir.AluOpType.is_equal,
                    )
                    nc.tensor.matmul(
                        psM[:, :], lhsT=selm[:, :], rhs=valcol[:, :],
                        start=True, stop=True,
                    )
                    nc.vector.tensor_copy(mrg[:, :], psM[:, :])
                    nc.vector.tensor_tensor(
                        out=lowm[:, :], in0=selm[:, :], in1=lt[:, :],
                        op=mybir.AluOpType.mult,
                    )
                    nc.vector.tensor_reduce(
                        out=cnt[:, :], in_=lowm[:, :], axis=mybir.AxisListType.X,
                        op=mybir.AluOpType.add,
                    )
                    nc.vector.tensor_scalar(
                        out=fmask[:, :], in0=cnt[:, :], scalar1=0.0, scalar2=None,
                        op0=mybir.AluOpType.is_equal,
                    )
                    nc.vector.select(
                        out=offs[:, :], mask=fmask[:, :],
                        on_true=idxcol[:, :], on_false=bigc[:, :],
                    )
                    nc.gpsimd.indirect_dma_start(
                        out=acc_d[:, :],
                        out_offset=bass.IndirectOffsetOnAxis(ap=offs[:, :], axis=0),
                        in_=mrg[:, :], in_offset=None,
                        compute_op=mybir.AluOpType.add,
                        bounds_check=N_NODES,
                        oob_is_err=False,
                    )

            tc.For_i_unrolled(0, COLS, 1, body, max_unroll=5)

            # ---- all-reduce accumulator + partials across the 8 cores ----
            nc.gpsimd.collective_compute(
                "AllReduce",
                mybir.AluOpType.add,
                replica_groups=[list(range(NCORES))],
                ins=[acc_d.ap().opt()],
                outs=[accr_d.ap().opt()],
            )
            nc.gpsimd.collective_compute(
                "AllReduce",
                mybir.AluOpType.add,
                replica_groups=[list(range(NCORES))],
                ins=[prt_d.ap().opt()],
                outs=[prtr_d.ap().opt()],
            )

            # ---- final loss ----
            nst = big.tile([P, ACC_C], f32, tag="nst")
            nc.sync.dma_start(out=nst[:, :], in_=accr_2d)
            ns2 = big.tile([P, ACC_C], f32, tag="ns2")
            nc.vector.tensor_tensor(
                out=ns2[:, :], in0=nst[:, :], in1=nst[:, :], op=mybir.AluOpType.mult
            )
            nc.vector.tensor_reduce(
                out=red[:, :], in_=ns2[:, :], axis=mybir.AxisListType.X,
                op=mybir.AluOpType.add,
            )
            kclp = ps.tile([1, 1], f32, tag="kclp")
            nc.tensor.matmul(
                kclp[:, :], lhsT=ones[:, :], rhs=red[:, :], start=True, stop=True
            )
            kcl = sm.tile([1, 1], f32, tag="kcl")
            nc.vector.tensor_scalar_mul(kcl[:, :], kclp[:, :], 1.0 / N_NODES)

            prtf = sm.tile([1, 8], f32, tag="prtf")
            nc.sync.dma_start(out=prtf[:, :], in_=prtr_d[:, :])
            # var = (s2 - s^2/E) / (E-1) per column; kvl = (varR + varX)/2
            E = float(N_EDGES)
            meanterm = sm.tile([1, 2], f32, tag="meanterm")
            s1 = prtf[:, :].rearrange("o (a b) -> o a b", b=2)[:, 0:2, 0]  # [1,2] sums
            s2 = prtf[:, :].rearrange("o (a b) -> o a b", b=2)[:, 0:2, 1]  # [1,2] sumsq
            nc.vector.tensor_tensor(
                out=meanterm[:, :], in0=s1, in1=s1, op=mybir.AluOpType.mult
            )
            nc.vector.tensor_scalar_mul(meanterm[:, :], meanterm[:, :], -1.0 / E)
            nc.vector.tensor_tensor(
                out=meanterm[:, :], in0=meanterm[:, :], in1=s2,
                op=mybir.AluOpType.add,
            )
            # sum the two variances: reduce [1,2] -> [1,1]
            kvl = sm.tile([1, 1], f32, tag="kvl")
            nc.vector.tensor_reduce(
                out=kvl[:, :], in_=meanterm[:, :], axis=mybir.AxisListType.X,
                op=mybir.AluOpType.add,
            )
            nc.vector.tensor_scalar_mul(kvl[:, :], kvl[:, :], 0.5 / (E - 1.0))

            res = sm.tile([1, 1], f32, tag="res")
            nc.vector.tensor_tensor(
                out=res[:, :], in0=kcl[:, :], in1=kvl[:, :], op=mybir.AluOpType.add
            )
            nc.sync.dma_start(out=out_d[:, :], in_=res[:, :])

    nc.compile()
    return nc


def kernel(node_features, edge_index, edge_logits, edge_params):
    from concourse.bass_utils import run_bass_kernel_spmd

    if "nc" not in _cache:
        _cache["nc"] = _build()
    nc = _cache["nc"]

    v = np.ascontiguousarray(node_features[:, 0:1], dtype=np.float32)
    src = np.asarray(edge_index[0], dtype=np.int32)
    dst = np.asarray(edge_index[1], dtype=np.int32)
    logits = np.asarray(edge_logits, dtype=np.float32)
    params = np.asarray(edge_params, dtype=np.float32)

    in_maps = []
    for k in range(NCORES):
        sl = slice(k * EPC, (k + 1) * EPC)
        in_maps.append({
            "v": v,
            "src": np.ascontiguousarray(src[sl].reshape(P, COLS)),
            "dst": np.ascontiguousarray(dst[sl].reshape(P, COLS)),
            "logits": np.ascontiguousarray(logits[sl].reshape(P, COLS)),
            "params": np.ascontiguousarray(params[sl].reshape(P, 2 * COLS)),
        })

    res = run_bass_kernel_spmd(nc, in_maps, core_ids=list(range(NCORES)))
    return np.float32(res.results[0]["out"][0, 0])



# revision 10
# speedup vs baseline: 2.6238x; 2.6238x over previous
"""Bass/Trainium2 kernel for nn_PhysicsLoss (GNN message passing physics loss).

Architecture (v13 "dual sorted-stream with mini-reduction"):

HW facts driving the design (measured on TRN2):
  * An indirect DMA carries exactly 128 offsets (one per SBUF partition row),
    each moving a contiguous window -> random access costs ~1.4-2.1 us per
    128 elements, serialized on the GPSIMD SWDGE descriptor generator.
  * Dense DVE/ACT ops, scans and dense DMAs are orders of magnitude cheaper.

So the kernel minimizes indirect-DMA instructions:
  * Two mirrored compact layouts per core: edges sorted by dst (layout D) and
    by src (layout S), nodes range-sharded across cores, each row of the
    [128, W] tile owning a contiguous slice of the sorted stream (node blocks
    never straddle rows; blocks padded to a multiple of 8 slots).
  * Each node block starts with a HEADER slot whose gather offset is the
    node itself; edge slots gather the opposite endpoint.  One window-DMA
    gather per column (W ~ 3.6K per layout instead of 2*3.1K+3.1K scatters).
  * A tensor_tensor_scan (state = hold*state + header_value) broadcasts the
    sorted-side node voltage across its block -> no second gather.
  * cur = |v_node - v_other| * w computed densely (w from logits/params,
    headers/pads get logits=-40, params=0 -> w ~ 4e-12, contributions ~0).
  * Node sums: dense 8:1 reduction of cur (strided adds) -> "minis", then
    one scatter-add per mini column (8x fewer scatter instructions).  Within
    a scatter column all 128 rows target disjoint node ranges -> race-free;
    consecutive columns rotate over 4 DRAM accumulator banks.
  * node_sum = (D banks) - (S banks), AllReduce over 8 cores, then
    kcl = mean(node_sum^2) plus the KVL variance term from all-reduced
    per-core partial sums (sum / sum-of-squares of edge_params).
"""
import numpy as np

N_NODES = 100000
N_EDGES = 3200000
NCORES = 8
P = 128
NPC = N_NODES // NCORES          # 12500 nodes per core (range shard)
ACC_ROWS = 100352                # 128 * 784 >= N_NODES
ACC_C = ACC_ROWS // P            # 784
PADROW = 100224                  # scatter/gather target for dead slots
EPS = 1e-6
PAD_LOGIT = -40.0                # sigmoid(-40) ~ 4e-18 -> w ~ 4e-12
NBANK = 4

# layout widths (columns per row, multiple of 8). Host asserts fit.
# slots/core ~ 12500 headers + 400K edges + pad-to-8 ~ 463K -> /128 ~ 3620
WD = 3720
WS = 3720

_cache = {}
_last_in_maps = None


def _build_layout(key, oth, logits, params, W):
    """Build per-core arrays for one sorted layout.

    key:   [E] node id each edge is grouped by (sorted side)
    oth:   [E] opposite endpoint node id (gathered side)
    Returns list over cores of dict(off, hold, hdrm, logits, params, moff).
    """
    E = key.shape[0]
    order = np.argsort(key, kind="stable")
    k_s = key[order]
    cores = []
    for c in range(NCORES):
        lo, hi = c * NPC, (c + 1) * NPC
        a = np.searchsorted(k_s, lo)
        b = np.searchsorted(k_s, hi)
        eidx = order[a:b]               # edges of this core, sorted by key
        kn = k_s[a:b]                   # their key node (sorted)
        # per-node degrees for nodes with >=1 edge
        nodes, counts = np.unique(kn, return_counts=True)
        nb = nodes.shape[0]
        bs = ((counts + 1 + 7) // 8) * 8          # block size (hdr+edges pad8)
        bstart = np.concatenate([[0], np.cumsum(bs)])  # [nb+1]
        total = int(bstart[-1])
        # assign blocks to 128 rows by near-equal slot count:
        # row(b) = #targets <= bstart[b], targets at total*i/128
        targets = (np.arange(1, P) * total) // P
        row_of_block = np.searchsorted(targets, bstart[:-1], side="right")
        rowlen = np.bincount(row_of_block, weights=bs, minlength=P).astype(np.int64)
        if nb and rowlen.max() > W:
            raise RuntimeError(f"row overflow {rowlen.max()} > {W}")
        rstart = np.concatenate([[0], np.cumsum(rowlen)])
        blocal = bstart[:-1] - rstart[row_of_block]   # block pos within row

        # slot arrays [P, W]
        off = np.full((P, W), PADROW, dtype=np.int32)
        hold = np.ones((P, W), dtype=np.float32)      # 1 - header_mask
        hdrm = np.zeros((P, W), dtype=np.float32)
        lg = np.full((P, W), PAD_LOGIT, dtype=np.float32)
        pr = np.zeros((P, W, 2), dtype=np.float32)
        moff = np.full((P, W // 8), PADROW, dtype=np.int32)

        # headers
        hr = row_of_block
        hc = blocal
        off[hr, hc] = nodes.astype(np.int32)
        hold[hr, hc] = 0.0
        hdrm[hr, hc] = 1.0
        # edges: rank within node
        rank = np.arange(b - a, dtype=np.int64) - \
            np.repeat(np.cumsum(np.concatenate([[0], counts[:-1]])), counts)
        er = np.repeat(row_of_block, counts)
        ec = np.repeat(blocal, counts) + 1 + rank
        off[er, ec] = oth[eidx].astype(np.int32)
        lg[er, ec] = logits[eidx]
        pr[er, ec] = params[eidx]
        # minis: each 8-group inside a block belongs to its node
        ng = bs // 8                                   # groups per block
        gr = np.repeat(row_of_block, ng)
        gc = np.repeat(blocal, ng) // 8 + \
            (np.arange(int(ng.sum()), dtype=np.int64) -
             np.repeat(np.cumsum(np.concatenate([[0], ng[:-1]])), ng))
        moff[gr, gc] = np.repeat(nodes, ng).astype(np.int32)

        cores.append({
            "off": off, "hold": hold, "hdrm": hdrm,
            "logits": lg, "params": pr.reshape(P, 2 * W), "moff": moff,
        })
    return cores


def _pack(edge_index, edge_logits, edge_params):
    src = np.asarray(edge_index[0], dtype=np.int64)
    dst = np.asarray(edge_index[1], dtype=np.int64)
    logits = np.asarray(edge_logits, dtype=np.float32)
    params = np.asarray(edge_params, dtype=np.float32)
    ld = _build_layout(dst, src, logits, params, WD)
    ls = _build_layout(src, dst, logits, params, WS)
    return ld, ls


def _build():
    import concourse.bass as bass
    import concourse.bacc as bacc
    import concourse.mybir as mybir
    from concourse.tile import TileContext

    f32 = mybir.dt.float32
    i32 = mybir.dt.int32

    nc = bacc.Bacc("TRN2", target_bir_lowering=False, debug=False, num_devices=NCORES)

    v_d = nc.dram_tensor("v", [ACC_ROWS, 1], f32, kind="ExternalInput")
    ins = {}
    for L, W in (("d", WD), ("s", WS)):
        ins[L] = {
            "off": nc.dram_tensor(f"off_{L}", [P, W], i32, kind="ExternalInput"),
            "hold": nc.dram_tensor(f"hold_{L}", [P, W], f32, kind="ExternalInput"),
            "hdrm": nc.dram_tensor(f"hdrm_{L}", [P, W], f32, kind="ExternalInput"),
            "logits": nc.dram_tensor(f"logits_{L}", [P, W], f32, kind="ExternalInput"),
            "params": nc.dram_tensor(f"params_{L}", [P, 2 * W], f32, kind="ExternalInput"),
            "moff": nc.dram_tensor(f"moff_{L}", [P, W // 8], i32, kind="ExternalInput"),
        }
    out_d = nc.dram_tensor("out", [1, 1], f32, kind="ExternalOutput")

    accD = [nc.dram_tensor(f"accD{i}", [ACC_ROWS, 1], f32) for i in range(NBANK)]
    accS = [nc.dram_tensor(f"accS{i}", [ACC_ROWS, 1], f32) for i in range(NBANK)]
    nsum_d = nc.dram_tensor("nsum", [ACC_ROWS, 1], f32)
    nsr_d = nc.dram_tensor("nsum_red", [ACC_ROWS, 1], f32)
    prt_d = nc.dram_tensor("prt_local", [1, 8], f32)
    prtr_d = nc.dram_tensor("prt_red", [1, 8], f32)

    acc2d = [a[:, :].rearrange("(p c) o -> p (c o)", p=P) for a in accD + accS]
    nsr_2d = nsr_d[:, :].rearrange("(p c) o -> p (c o)", p=P)

    with TileContext(nc) as tc:
        with (
            tc.tile_pool(name="sm", bufs=1) as sm,
            tc.tile_pool(name="ps", bufs=2, space="PSUM") as ps,
        ):
            # ---- zero accumulator banks ----
            zt = sm.tile([P, ACC_C], f32, tag="zt")
            nc.vector.memset(zt[:, :], 0.0)
            for a2 in acc2d:
                nc.sync.dma_start(out=a2, in_=zt[:, :])

            red = sm.tile([P, 1], f32, tag="red")
            ones = sm.tile([P, 1], f32, tag="ones")
            nc.vector.memset(ones[:, :], 1.0)
            prt = sm.tile([1, 8], f32, tag="prt")
            nc.vector.memset(prt[:, :], 0.0)
            pssc = ps.tile([1, 1], f32, tag="pssc")

            with tc.tile_pool(name="lay", bufs=1) as lay:
                for L, W, banks in (("d", WD, accD), ("s", WS, accS)):
                    io = ins[L]
                    # tags shared across the two layout passes -> SBUF reuse
                    offt = lay.tile([P, W], i32, tag="offt")
                    nc.sync.dma_start(out=offt[:, :], in_=io["off"][:, :])
                    holdt = lay.tile([P, W], f32, tag="holdt")
                    nc.sync.dma_start(out=holdt[:, :], in_=io["hold"][:, :])
                    hdrt = lay.tile([P, W], f32, tag="hdrt")
                    nc.sync.dma_start(out=hdrt[:, :], in_=io["hdrm"][:, :])
                    logt = lay.tile([P, W], f32, tag="logt")
                    nc.sync.dma_start(out=logt[:, :], in_=io["logits"][:, :])
                    part = lay.tile([P, 2 * W], f32, tag="part")
                    nc.sync.dma_start(out=part[:, :], in_=io["params"][:, :])
                    mofft = lay.tile([P, W // 8], i32, tag="mofft")
                    nc.sync.dma_start(out=mofft[:, :], in_=io["moff"][:, :])

                    # dense weight: w = sigmoid(logit) / (R+X+eps); wt aliases imp
                    par3 = part[:, :].rearrange("p (c two) -> p c two", two=2)
                    imp = lay.tile([P, W], f32, tag="imp")
                    nc.vector.tensor_tensor(
                        out=imp[:, :], in0=par3[:, :, 0], in1=par3[:, :, 1],
                        op=mybir.AluOpType.add,
                    )
                    nc.vector.tensor_scalar_add(imp[:, :], imp[:, :], EPS)
                    nc.vector.reciprocal(imp[:, :], imp[:, :])
                    sig = lay.tile([P, W], f32, tag="sig")
                    nc.scalar.activation(
                        sig[:, :], logt[:, :], mybir.ActivationFunctionType.Sigmoid
                    )
                    wt = imp  # in-place: w = sig * (1/imp)
                    nc.vector.tensor_tensor(
                        out=wt[:, :], in0=sig[:, :], in1=imp[:, :],
                        op=mybir.AluOpType.mult,
                    )

                    # KVL partial sums from layout D only (headers/pads have
                    # params == 0 and contribute nothing)
                    if L == "d":
                        sq = logt  # logits no longer needed
                        for k in range(4):  # R, R^2, X, X^2
                            colap = par3[:, :, k // 2]
                            if k % 2 == 0:
                                nc.vector.tensor_reduce(
                                    out=red[:, :], in_=colap,
                                    axis=mybir.AxisListType.X, op=mybir.AluOpType.add,
                                )
                            else:
                                nc.vector.tensor_tensor(
                                    out=sq[:, :], in0=colap, in1=colap,
                                    op=mybir.AluOpType.mult,
                                )
                                nc.vector.tensor_reduce(
                                    out=red[:, :], in_=sq[:, :],
                                    axis=mybir.AxisListType.X, op=mybir.AluOpType.add,
                                )
                            nc.tensor.matmul(
                                pssc[:, :], lhsT=ones[:, :], rhs=red[:, :],
                                start=True, stop=True,
                            )
                            nc.vector.tensor_copy(prt[:, k:k + 1], pssc[:, :])
                        nc.sync.dma_start(out=prt_d[:, :], in_=prt[:, :])

                    # ---- gather loop: one window-DMA per column ----
                    gt = lay.tile([P, W], f32, tag="gt")
                    for c2 in range(W):
                        nc.gpsimd.indirect_dma_start(
                            out=gt[:, bass.ds(c2, 1)], out_offset=None,
                            in_=v_d[:, :],
                            in_offset=bass.IndirectOffsetOnAxis(
                                ap=offt[:, bass.ds(c2, 1)], axis=0),
                        )

                    # ---- broadcast sorted-side node voltage via reset-scan ----
                    hv = sig  # sigmoid no longer needed
                    nc.vector.tensor_tensor(
                        out=hv[:, :], in0=gt[:, :], in1=hdrt[:, :],
                        op=mybir.AluOpType.mult,
                    )
                    vb = lay.tile([P, W], f32, tag="vb")
                    nc.vector.tensor_tensor_scan(
                        out=vb[:, :], data0=holdt[:, :], data1=hv[:, :],
                        initial=0.0, op0=mybir.AluOpType.mult,
                        op1=mybir.AluOpType.add,
                    )

                    # ---- cur = |vb - g| * w (in place over vb) ----
                    cur = vb
                    nc.vector.tensor_tensor(
                        out=cur[:, :], in0=vb[:, :], in1=gt[:, :],
                        op=mybir.AluOpType.subtract,
                    )
                    nc.scalar.activation(
                        cur[:, :], cur[:, :], mybir.ActivationFunctionType.Abs
                    )
                    nc.vector.tensor_tensor(
                        out=cur[:, :], in0=cur[:, :], in1=wt[:, :],
                        op=mybir.AluOpType.mult,
                    )

                    # ---- minis: 8:1 reduction along the stream ----
                    cur3 = cur[:, :].rearrange("p (m e) -> p m e", e=8)
                    mini = lay.tile([P, W // 8], f32, tag="mini")
                    nc.vector.tensor_tensor(
                        out=mini[:, :], in0=cur3[:, :, 0], in1=cur3[:, :, 1],
                        op=mybir.AluOpType.add,
                    )
                    for e in range(2, 8):
                        nc.vector.tensor_tensor(
                            out=mini[:, :], in0=mini[:, :], in1=cur3[:, :, e],
                            op=mybir.AluOpType.add,
                        )

                    # ---- mini scatter-add, rotating banks ----
                    for mc in range(W // 8):
                        nc.gpsimd.indirect_dma_start(
                            out=banks[mc % NBANK][:, :],
                            out_offset=bass.IndirectOffsetOnAxis(
                                ap=mofft[:, bass.ds(mc, 1)], axis=0),
                            in_=mini[:, bass.ds(mc, 1)], in_offset=None,
                            compute_op=mybir.AluOpType.add,
                        )

            with tc.tile_pool(name="fin", bufs=1) as fin:
                # ---- node_sum = sum(accD) - sum(accS) ----
                at = [
                    fin.tile([P, ACC_C], f32, tag=f"acct{i}", name=f"acct{i}")
                    for i in range(2 * NBANK)
                ]
                for i in range(2 * NBANK):
                    nc.sync.dma_start(out=at[i][:, :], in_=acc2d[i])
                nsm = fin.tile([P, ACC_C], f32, tag="nsm")
                nc.vector.tensor_tensor(
                    out=nsm[:, :], in0=at[0][:, :], in1=at[1][:, :],
                    op=mybir.AluOpType.add,
                )
                for i in range(2, NBANK):
                    nc.vector.tensor_tensor(
                        out=nsm[:, :], in0=nsm[:, :], in1=at[i][:, :],
                        op=mybir.AluOpType.add,
                    )
                for i in range(NBANK, 2 * NBANK):
                    nc.vector.tensor_tensor(
                        out=nsm[:, :], in0=nsm[:, :], in1=at[i][:, :],
                        op=mybir.AluOpType.subtract,
                    )
                nsum_2d = nsum_d[:, :].rearrange("(p c) o -> p (c o)", p=P)
                nc.sync.dma_start(out=nsum_2d, in_=nsm[:, :])

                # ---- all-reduce across cores ----
                nc.gpsimd.collective_compute(
                    "AllReduce", mybir.AluOpType.add,
                    replica_groups=[list(range(NCORES))],
                    ins=[nsum_d.ap().opt()], outs=[nsr_d.ap().opt()],
                )
                nc.gpsimd.collective_compute(
                    "AllReduce", mybir.AluOpType.add,
                    replica_groups=[list(range(NCORES))],
                    ins=[prt_d.ap().opt()], outs=[prtr_d.ap().opt()],
                )

                # ---- final loss ----
                nst = fin.tile([P, ACC_C], f32, tag="nst")
                nc.sync.dma_start(out=nst[:, :], in_=nsr_2d)
                ns2 = fin.tile([P, ACC_C], f32, tag="ns2")
                nc.vector.tensor_tensor(
                    out=ns2[:, :], in0=nst[:, :], in1=nst[:, :],
                    op=mybir.AluOpType.mult,
                )
                nc.vector.tensor_reduce(
                    out=red[:, :], in_=ns2[:, :], axis=mybir.AxisListType.X,
                    op=mybir.AluOpType.add,
                )
                kclp = ps.tile([1, 1], f32, tag="kclp")
                nc.tensor.matmul(
                    kclp[:, :], lhsT=ones[:, :], rhs=red[:, :], start=True, stop=True
                )
                kcl = sm.tile([1, 1], f32, tag="kcl")
                nc.vector.tensor_scalar_mul(kcl[:, :], kclp[:, :], 1.0 / N_NODES)

                prtf = sm.tile([1, 8], f32, tag="prtf")
                nc.sync.dma_start(out=prtf[:, :], in_=prtr_d[:, :])
                E = float(N_EDGES)
                meanterm = sm.tile([1, 2], f32, tag="meanterm")
                s1 = prtf[:, :].rearrange("o (a b) -> o a b", b=2)[:, 0:2, 0]
                s2 = prtf[:, :].rearrange("o (a b) -> o a b", b=2)[:, 0:2, 1]
                nc.vector.tensor_tensor(
                    out=meanterm[:, :], in0=s1, in1=s1, op=mybir.AluOpType.mult
                )
                nc.vector.tensor_scalar_mul(meanterm[:, :], meanterm[:, :], -1.0 / E)
                nc.vector.tensor_tensor(
                    out=meanterm[:, :], in0=meanterm[:, :], in1=s2,
                    op=mybir.AluOpType.add,
                )
                kvl = sm.tile([1, 1], f32, tag="kvl")
                nc.vector.tensor_reduce(
                    out=kvl[:, :], in_=meanterm[:, :], axis=mybir.AxisListType.X,
                    op=mybir.AluOpType.add,
                )
                nc.vector.tensor_scalar_mul(kvl[:, :], kvl[:, :], 0.5 / (E - 1.0))

                res = sm.tile([1, 1], f32, tag="res")
                nc.vector.tensor_tensor(
                    out=res[:, :], in0=kcl[:, :], in1=kvl[:, :],
                    op=mybir.AluOpType.add,
                )
                nc.sync.dma_start(out=out_d[:, :], in_=res[:, :])

    nc.compile()
    return nc


def kernel(node_features, edge_index, edge_logits, edge_params):
    global _last_in_maps
    from concourse.bass_utils import run_bass_kernel_spmd

    if "nc" not in _cache:
        _cache["nc"] = _build()
    nc = _cache["nc"]

    v = np.zeros((ACC_ROWS, 1), dtype=np.float32)
    v[:N_NODES, 0] = np.asarray(node_features[:, 0], dtype=np.float32)
    ld, ls = _pack(edge_index, edge_logits, edge_params)

    in_maps = []
    for c in range(NCORES):
        m = {"v": v}
        for L, lay in (("d", ld), ("s", ls)):
            for k in ("off", "hold", "hdrm", "logits", "params", "moff"):
                m[f"{k}_{L}"] = np.ascontiguousarray(lay[c][k])
        in_maps.append(m)
    _last_in_maps = in_maps

    res = run_bass_kernel_spmd(nc, in_maps, core_ids=list(range(NCORES)))
    return np.float32(res.results[0]["out"][0, 0])


# revision 14
# speedup vs baseline: 3.4782x; 1.3256x over previous
"""Bass/Trainium2 kernel for nn_PhysicsLoss (GNN message passing physics loss).

Architecture (v13 "dual sorted-stream with mini-reduction"):

HW facts driving the design (measured on TRN2):
  * An indirect DMA carries exactly 128 offsets (one per SBUF partition row),
    each moving a contiguous window -> random access costs ~1.4-2.1 us per
    128 elements, serialized on the GPSIMD SWDGE descriptor generator.
  * Dense DVE/ACT ops, scans and dense DMAs are orders of magnitude cheaper.

So the kernel minimizes indirect-DMA instructions:
  * Two mirrored compact layouts per core: edges sorted by dst (layout D) and
    by src (layout S), nodes range-sharded across cores, each row of the
    [128, W] tile owning a contiguous slice of the sorted stream (node blocks
    never straddle rows; blocks padded to a multiple of 8 slots).
  * Each node block starts with a HEADER slot whose gather offset is the
    node itself; edge slots gather the opposite endpoint.  One window-DMA
    gather per column (W ~ 3.6K per layout instead of 2*3.1K+3.1K scatters).
  * A tensor_tensor_scan (state = hold*state + header_value) broadcasts the
    sorted-side node voltage across its block -> no second gather.
  * cur = |v_node - v_other| * w computed densely (w from logits/params,
    headers/pads get logits=-40, params=0 -> w ~ 4e-12, contributions ~0).
  * Node sums: dense 8:1 reduction of cur (strided adds) -> "minis", then
    one scatter-add per mini column (8x fewer scatter instructions).  Within
    a scatter column all 128 rows target disjoint node ranges -> race-free;
    consecutive columns rotate over 4 DRAM accumulator banks.
  * node_sum = (D banks) - (S banks), AllReduce over 8 cores, then
    kcl = mean(node_sum^2) plus the KVL variance term from all-reduced
    per-core partial sums (sum / sum-of-squares of edge_params).
"""
import numpy as np

N_NODES = 100000
N_EDGES = 3200000
NCORES = 8
P = 128
NPC = N_NODES // NCORES          # 12500 nodes per core (range shard)
ACC_ROWS = 100352                # 128 * 784 >= N_NODES
ACC_C = ACC_ROWS // P            # 784
PADROW = 100224                  # scatter/gather target for dead slots
EPS = 1e-6
PAD_LOGIT = -40.0                # sigmoid(-40) ~ 4e-18 -> w ~ 4e-12
NBANK = 1

# layout widths (columns per row). Host asserts fit.
# compact slots/core ~ 12500 headers + 400K edges -> max balanced row ~ 3265
WD = 3280
WS = 3280
KMAX = 112                       # max nodes per row (measured 103) + margin

_cache = {}
_last_in_maps = None


def _build_layout(key, oth, logits, params, W):
    """Build per-core arrays for one sorted layout (compact blocks).

    Returns list over cores of dict(off, hold, hdrm, logits, params,
    offx, fnode).  Rows are parity-interleaved over physical partitions so
    the two final window-scatters (partitions 0-63 = even logical rows,
    64-127 = odd) have non-overlapping node windows.
    """
    E = key.shape[0]
    order = np.argsort(key, kind="stable")
    k_s = key[order]
    cores = []
    for c in range(NCORES):
        lo, hi = c * NPC, (c + 1) * NPC
        a = np.searchsorted(k_s, lo)
        b = np.searchsorted(k_s, hi)
        eidx = order[a:b]               # edges of this core, sorted by key
        kn = k_s[a:b]
        nodes, counts = np.unique(kn, return_counts=True)
        nb = nodes.shape[0]
        if nb != NPC:
            raise RuntimeError("every node must have >=1 edge per side")
        bs = counts + 1                                # header + edges
        bstart = np.concatenate([[0], np.cumsum(bs)])
        total = int(bstart[-1])
        # balanced assignment of blocks to 128 logical rows
        targets = (np.arange(1, P) * total) // P
        lrow = np.searchsorted(targets, bstart[:-1], side="right")
        rowlen = np.bincount(lrow, weights=bs, minlength=P).astype(np.int64)
        if rowlen.max() > W:
            raise RuntimeError(f"row overflow {rowlen.max()} > {W}")
        nodecnt = np.bincount(lrow, minlength=P)
        if nodecnt.max() > KMAX:
            raise RuntimeError(f"nodes/row overflow {nodecnt.max()} > {KMAX}")
        if nodecnt.min() + np.sort(nodecnt)[1] < KMAX:
            # adjacent-row windows must not reach past the next-next row
            raise RuntimeError("KMAX too large for window scatter safety")
        # parity interleave: logical row l -> physical partition
        prow_of_l = np.where(np.arange(P) % 2 == 0,
                             np.arange(P) // 2, 64 + np.arange(P) // 2)
        prow = prow_of_l[lrow]                          # per block
        rstart = np.concatenate([[0], np.cumsum(rowlen)])
        blocal = bstart[:-1] - rstart[lrow]             # block pos within row

        off = np.full((P, W), PADROW, dtype=np.int32)
        hold = np.ones((P, W), dtype=np.float32)
        hdrm = np.zeros((P, W), dtype=np.float32)
        lg = np.full((P, W), PAD_LOGIT, dtype=np.float32)
        pr = np.zeros((P, W, 2), dtype=np.float32)

        # headers
        off[prow, blocal] = nodes.astype(np.int32)
        hold[prow, blocal] = 0.0
        hdrm[prow, blocal] = 1.0
        # edges
        rank = np.arange(b - a, dtype=np.int64) - \
            np.repeat(np.cumsum(np.concatenate([[0], counts[:-1]])), counts)
        er = np.repeat(prow, counts)
        ec = np.repeat(blocal, counts) + 1 + rank
        off[er, ec] = oth[eidx].astype(np.int32)
        lg[er, ec] = logits[eidx]
        pr[er, ec] = params[eidx]

        # extraction offsets: inclusive prefix position of each block end,
        # in global [P*W] coordinates of the prefix DRAM buffer
        endpos = blocal + bs - 1                        # last slot of block
        k_in_row = np.zeros(nb, dtype=np.int64)
        first_of_row = np.concatenate([[0], np.cumsum(nodecnt)])[:-1]
        k_in_row = np.arange(nb) - first_of_row[lrow]
        offx = np.zeros((P, KMAX), dtype=np.int32)
        # default: every k points at slot 0 of its row; real entries below,
        # then forward-fill tail with the last real end position
        offx[:] = (np.arange(P)[:, None] * W).astype(np.int32)
        offx[prow, k_in_row] = (prow * W + endpos).astype(np.int32)
        cnt_p = np.zeros(P, dtype=np.int64)
        cnt_p[prow_of_l] = nodecnt                      # per physical row
        for p in range(P):
            if cnt_p[p] < KMAX:
                offx[p, cnt_p[p]:] = offx[p, cnt_p[p] - 1]
        # hmm: k=0 entries must diff against 0 -> handled by shifted tile
        firstnode = np.zeros(P, dtype=np.int64)
        firstnode[prow_of_l] = lo + np.concatenate([[0], np.cumsum(nodecnt)])[:-1]
        lrow_of_p = np.zeros(P, dtype=np.int64)
        lrow_of_p[prow_of_l] = np.arange(P)
        fnode = np.zeros((P, 2), dtype=np.int32)
        pmask = np.zeros((P, 2), dtype=np.float32)
        for h in range(2):
            act = (lrow_of_p % 2) == h
            fnode[:, h] = np.where(act, firstnode, PADROW).astype(np.int32)
            pmask[:, h] = act.astype(np.float32)

        cores.append({
            "off": off, "hold": hold, "hdrm": hdrm,
            "logits": lg, "params": pr.reshape(P, 2 * W),
            "offx": offx, "fnode": fnode, "pmask": pmask,
        })
    return cores


def _pack(edge_index, edge_logits, edge_params):
    src = np.asarray(edge_index[0], dtype=np.int64)
    dst = np.asarray(edge_index[1], dtype=np.int64)
    logits = np.asarray(edge_logits, dtype=np.float32)
    params = np.asarray(edge_params, dtype=np.float32)
    ld = _build_layout(dst, src, logits, params, WD)
    ls = _build_layout(src, dst, logits, params, WS)
    return ld, ls


def _build():
    import concourse.bass as bass
    import concourse.bacc as bacc
    import concourse.mybir as mybir
    from concourse.tile import TileContext

    f32 = mybir.dt.float32
    i32 = mybir.dt.int32
    KM = KMAX

    nc = bacc.Bacc("TRN2", target_bir_lowering=False, debug=False, num_devices=NCORES)

    v_d = nc.dram_tensor("v", [ACC_ROWS, 1], f32, kind="ExternalInput")
    ins = {}
    for L, W in (("d", WD), ("s", WS)):
        ins[L] = {
            "off": nc.dram_tensor(f"off_{L}", [P, W], i32, kind="ExternalInput"),
            "hold": nc.dram_tensor(f"hold_{L}", [P, W], f32, kind="ExternalInput"),
            "hdrm": nc.dram_tensor(f"hdrm_{L}", [P, W], f32, kind="ExternalInput"),
            "logits": nc.dram_tensor(f"logits_{L}", [P, W], f32, kind="ExternalInput"),
            "params": nc.dram_tensor(f"params_{L}", [P, 2 * W], f32, kind="ExternalInput"),
            "offx": nc.dram_tensor(f"offx_{L}", [P, KMAX], i32, kind="ExternalInput"),
            "fnode": nc.dram_tensor(f"fnode_{L}", [P, 2], i32, kind="ExternalInput"),
            "pmask": nc.dram_tensor(f"pmask_{L}", [P, 2], f32, kind="ExternalInput"),
        }
    out_d = nc.dram_tensor("out", [1, 1], f32, kind="ExternalOutput")

    accD = [nc.dram_tensor(f"accD{i}", [ACC_ROWS, 1], f32) for i in range(NBANK)]
    accS = [nc.dram_tensor(f"accS{i}", [ACC_ROWS, 1], f32) for i in range(NBANK)]
    nsum_d = nc.dram_tensor("nsum", [ACC_ROWS, 1], f32)
    nsr_d = nc.dram_tensor("nsum_red", [ACC_ROWS, 1], f32)
    prt_d = nc.dram_tensor("prt_local", [1, 8], f32)
    prtr_d = nc.dram_tensor("prt_red", [1, 8], f32)
    pfx_d = nc.dram_tensor("pfx", [P * WD, 1], f32)

    acc2d = [a[:, :].rearrange("(p c) o -> p (c o)", p=P) for a in accD + accS]
    nsr_2d = nsr_d[:, :].rearrange("(p c) o -> p (c o)", p=P)

    with TileContext(nc) as tc:
        with (
            tc.tile_pool(name="sm", bufs=1) as sm,
            tc.tile_pool(name="ps", bufs=2, space="PSUM") as ps,
        ):
            # ---- zero accumulator banks ----
            zt = sm.tile([P, ACC_C], f32, tag="zt")
            nc.vector.memset(zt[:, :], 0.0)
            for a2 in acc2d:
                nc.sync.dma_start(out=a2, in_=zt[:, :])

            red = sm.tile([P, 1], f32, tag="red")
            ones = sm.tile([P, 1], f32, tag="ones")
            nc.vector.memset(ones[:, :], 1.0)
            prt = sm.tile([1, 8], f32, tag="prt")
            nc.vector.memset(prt[:, :], 0.0)
            pssc = ps.tile([1, 1], f32, tag="pssc")

            with tc.tile_pool(name="lay", bufs=1) as lay:
                for L, W, banks in (("d", WD, accD), ("s", WS, accS)):
                    io = ins[L]
                    # tags shared across the two layout passes -> SBUF reuse
                    offt = lay.tile([P, W], i32, tag="offt")
                    nc.sync.dma_start(out=offt[:, :], in_=io["off"][:, :])
                    holdt = lay.tile([P, W], f32, tag="holdt")
                    nc.sync.dma_start(out=holdt[:, :], in_=io["hold"][:, :])
                    hdrt = lay.tile([P, W], f32, tag="hdrt")
                    nc.sync.dma_start(out=hdrt[:, :], in_=io["hdrm"][:, :])
                    logt = lay.tile([P, W], f32, tag="logt")
                    nc.sync.dma_start(out=logt[:, :], in_=io["logits"][:, :])
                    part = lay.tile([P, 2 * W], f32, tag="part")
                    nc.sync.dma_start(out=part[:, :], in_=io["params"][:, :])
                    offxt = lay.tile([P, KM], i32, tag="offxt")
                    nc.sync.dma_start(out=offxt[:, :], in_=io["offx"][:, :])
                    fnt = lay.tile([P, 2], i32, tag="fnt")
                    nc.sync.dma_start(out=fnt[:, :], in_=io["fnode"][:, :])
                    pmk = lay.tile([P, 2], f32, tag="pmk")
                    nc.sync.dma_start(out=pmk[:, :], in_=io["pmask"][:, :])

                    # dense weight: w = sigmoid(logit) / (R+X+eps); wt aliases imp
                    par3 = part[:, :].rearrange("p (c two) -> p c two", two=2)
                    imp = lay.tile([P, W], f32, tag="imp")
                    nc.vector.tensor_tensor(
                        out=imp[:, :], in0=par3[:, :, 0], in1=par3[:, :, 1],
                        op=mybir.AluOpType.add,
                    )
                    nc.vector.tensor_scalar_add(imp[:, :], imp[:, :], EPS)
                    nc.vector.reciprocal(imp[:, :], imp[:, :])
                    sig = lay.tile([P, W], f32, tag="sig")
                    nc.scalar.activation(
                        sig[:, :], logt[:, :], mybir.ActivationFunctionType.Sigmoid
                    )
                    wt = imp  # in-place: w = sig * (1/imp)
                    nc.vector.tensor_tensor(
                        out=wt[:, :], in0=sig[:, :], in1=imp[:, :],
                        op=mybir.AluOpType.mult,
                    )

                    # KVL partial sums from layout D only (headers/pads have
                    # params == 0 and contribute nothing)
                    if L == "d":
                        sq = logt  # logits no longer needed
                        for k in range(4):  # R, R^2, X, X^2
                            colap = par3[:, :, k // 2]
                            if k % 2 == 0:
                                nc.vector.tensor_reduce(
                                    out=red[:, :], in_=colap,
                                    axis=mybir.AxisListType.X, op=mybir.AluOpType.add,
                                )
                            else:
                                nc.vector.tensor_tensor(
                                    out=sq[:, :], in0=colap, in1=colap,
                                    op=mybir.AluOpType.mult,
                                )
                                nc.vector.tensor_reduce(
                                    out=red[:, :], in_=sq[:, :],
                                    axis=mybir.AxisListType.X, op=mybir.AluOpType.add,
                                )
                            nc.tensor.matmul(
                                pssc[:, :], lhsT=ones[:, :], rhs=red[:, :],
                                start=True, stop=True,
                            )
                            nc.vector.tensor_copy(prt[:, k:k + 1], pssc[:, :])
                        nc.sync.dma_start(out=prt_d[:, :], in_=prt[:, :])

                    # ---- gather loop: one window-DMA per column ----
                    gt = lay.tile([P, W], f32, tag="gt")
                    for c2 in range(W):
                        nc.gpsimd.indirect_dma_start(
                            out=gt[:, bass.ds(c2, 1)], out_offset=None,
                            in_=v_d[:, :],
                            in_offset=bass.IndirectOffsetOnAxis(
                                ap=offt[:, bass.ds(c2, 1)], axis=0),
                        )

                    # ---- broadcast sorted-side node voltage via reset-scan ----
                    hv = sig  # sigmoid no longer needed
                    nc.vector.tensor_tensor(
                        out=hv[:, :], in0=gt[:, :], in1=hdrt[:, :],
                        op=mybir.AluOpType.mult,
                    )
                    vb = lay.tile([P, W], f32, tag="vb")
                    nc.vector.tensor_tensor_scan(
                        out=vb[:, :], data0=holdt[:, :], data1=hv[:, :],
                        initial=0.0, op0=mybir.AluOpType.mult,
                        op1=mybir.AluOpType.add,
                    )

                    # ---- cur = |vb - g| * w (in place over vb) ----
                    cur = vb
                    nc.vector.tensor_tensor(
                        out=cur[:, :], in0=vb[:, :], in1=gt[:, :],
                        op=mybir.AluOpType.subtract,
                    )
                    nc.scalar.activation(
                        cur[:, :], cur[:, :], mybir.ActivationFunctionType.Abs
                    )
                    nc.vector.tensor_tensor(
                        out=cur[:, :], in0=cur[:, :], in1=wt[:, :],
                        op=mybir.AluOpType.mult,
                    )

                    # ---- full-row inclusive prefix sum of cur ----
                    onw = sig  # reuse as an all-ones tile
                    nc.vector.memset(onw[:, :], 1.0)
                    pfx = gt  # reuse: gathered values no longer needed
                    nc.vector.tensor_tensor_scan(
                        out=pfx[:, :], data0=onw[:, :],
                        data1=cur[:, :], initial=0.0,
                        op0=mybir.AluOpType.mult, op1=mybir.AluOpType.add,
                    )
                    pfx2d = pfx_d[:, :].rearrange("(p w) o -> p (w o)", p=P)
                    nc.sync.dma_start(out=pfx2d[:, :W], in_=pfx[:, :])

                    # ---- extract per-node block-end prefix values ----
                    et = lay.tile([P, KM], f32, tag="et")
                    for k in range(KM):
                        nc.gpsimd.indirect_dma_start(
                            out=et[:, bass.ds(k, 1)], out_offset=None,
                            in_=pfx_d[:, :],
                            in_offset=bass.IndirectOffsetOnAxis(
                                ap=offxt[:, bass.ds(k, 1)], axis=0),
                        )
                    # totals: T[p, k] = E[p, k] - E[p, k-1] (E[p, -1] = 0)
                    sh = lay.tile([P, KM], f32, tag="sh")
                    nc.vector.memset(sh[:, :1], 0.0)
                    nc.vector.tensor_copy(sh[:, 1:], et[:, :KM - 1])
                    tt = lay.tile([P, KM], f32, tag="tt")
                    nc.vector.tensor_tensor(
                        out=tt[:, :], in0=et[:, :], in1=sh[:, :],
                        op=mybir.AluOpType.subtract,
                    )
                    # ---- two window scatter-adds (parity-disjoint rows;
                    # inactive parity rows add zeros into the pad region) ----
                    ttm = sh  # reuse
                    for h in range(2):
                        nc.vector.tensor_scalar(
                            out=ttm[:, :], in0=tt[:, :],
                            scalar1=pmk[:, h:h + 1], scalar2=None,
                            op0=mybir.AluOpType.mult,
                        )
                        nc.gpsimd.indirect_dma_start(
                            out=banks[0][:, :],
                            out_offset=bass.IndirectOffsetOnAxis(
                                ap=fnt[:, h:h + 1], axis=0),
                            in_=ttm[:, :], in_offset=None,
                            compute_op=mybir.AluOpType.add,
                        )

            with tc.tile_pool(name="fin", bufs=1) as fin:
                # ---- node_sum = sum(accD) - sum(accS) ----
                at = [
                    fin.tile([P, ACC_C], f32, tag=f"acct{i}", name=f"acct{i}")
                    for i in range(2 * NBANK)
                ]
                for i in range(2 * NBANK):
                    nc.sync.dma_start(out=at[i][:, :], in_=acc2d[i])
                nsm = fin.tile([P, ACC_C], f32, tag="nsm")
                nc.vector.tensor_tensor(
                    out=nsm[:, :], in0=at[0][:, :], in1=at[NBANK][:, :],
                    op=mybir.AluOpType.subtract,
                )
                for i in range(1, NBANK):
                    nc.vector.tensor_tensor(
                        out=nsm[:, :], in0=nsm[:, :], in1=at[i][:, :],
                        op=mybir.AluOpType.add,
                    )
                for i in range(NBANK + 1, 2 * NBANK):
                    nc.vector.tensor_tensor(
                        out=nsm[:, :], in0=nsm[:, :], in1=at[i][:, :],
                        op=mybir.AluOpType.subtract,
                    )
                nsum_2d = nsum_d[:, :].rearrange("(p c) o -> p (c o)", p=P)
                nc.sync.dma_start(out=nsum_2d, in_=nsm[:, :])

                # ---- all-reduce across cores ----
                nc.gpsimd.collective_compute(
                    "AllReduce", mybir.AluOpType.add,
                    replica_groups=[list(range(NCORES))],
                    ins=[nsum_d.ap().opt()], outs=[nsr_d.ap().opt()],
                )
                nc.gpsimd.collective_compute(
                    "AllReduce", mybir.AluOpType.add,
                    replica_groups=[list(range(NCORES))],
                    ins=[prt_d.ap().opt()], outs=[prtr_d.ap().opt()],
                )

                # ---- final loss ----
                nst = fin.tile([P, ACC_C], f32, tag="nst")
                nc.sync.dma_start(out=nst[:, :], in_=nsr_2d)
                ns2 = fin.tile([P, ACC_C], f32, tag="ns2")
                nc.vector.tensor_tensor(
                    out=ns2[:, :], in0=nst[:, :], in1=nst[:, :],
                    op=mybir.AluOpType.mult,
                )
                nc.vector.tensor_reduce(
                    out=red[:, :], in_=ns2[:, :], axis=mybir.AxisListType.X,
                    op=mybir.AluOpType.add,
                )
                kclp = ps.tile([1, 1], f32, tag="kclp")
                nc.tensor.matmul(
                    kclp[:, :], lhsT=ones[:, :], rhs=red[:, :], start=True, stop=True
                )
                kcl = sm.tile([1, 1], f32, tag="kcl")
                nc.vector.tensor_scalar_mul(kcl[:, :], kclp[:, :], 1.0 / N_NODES)

                prtf = sm.tile([1, 8], f32, tag="prtf")
                nc.sync.dma_start(out=prtf[:, :], in_=prtr_d[:, :])
                E = float(N_EDGES)
                meanterm = sm.tile([1, 2], f32, tag="meanterm")
                s1 = prtf[:, :].rearrange("o (a b) -> o a b", b=2)[:, 0:2, 0]
                s2 = prtf[:, :].rearrange("o (a b) -> o a b", b=2)[:, 0:2, 1]
                nc.vector.tensor_tensor(
                    out=meanterm[:, :], in0=s1, in1=s1, op=mybir.AluOpType.mult
                )
                nc.vector.tensor_scalar_mul(meanterm[:, :], meanterm[:, :], -1.0 / E)
                nc.vector.tensor_tensor(
                    out=meanterm[:, :], in0=meanterm[:, :], in1=s2,
                    op=mybir.AluOpType.add,
                )
                kvl = sm.tile([1, 1], f32, tag="kvl")
                nc.vector.tensor_reduce(
                    out=kvl[:, :], in_=meanterm[:, :], axis=mybir.AxisListType.X,
                    op=mybir.AluOpType.add,
                )
                nc.vector.tensor_scalar_mul(kvl[:, :], kvl[:, :], 0.5 / (E - 1.0))

                res = sm.tile([1, 1], f32, tag="res")
                nc.vector.tensor_tensor(
                    out=res[:, :], in0=kcl[:, :], in1=kvl[:, :],
                    op=mybir.AluOpType.add,
                )
                nc.sync.dma_start(out=out_d[:, :], in_=res[:, :])

    nc.compile()
    return nc


def kernel(node_features, edge_index, edge_logits, edge_params):
    global _last_in_maps
    from concourse.bass_utils import run_bass_kernel_spmd

    if "nc" not in _cache:
        _cache["nc"] = _build()
    nc = _cache["nc"]

    v = np.zeros((ACC_ROWS, 1), dtype=np.float32)
    v[:N_NODES, 0] = np.asarray(node_features[:, 0], dtype=np.float32)
    ld, ls = _pack(edge_index, edge_logits, edge_params)

    in_maps = []
    for c in range(NCORES):
        m = {"v": v}
        for L, lay in (("d", ld), ("s", ls)):
            for k in ("off", "hold", "hdrm", "logits", "params", "offx", "fnode", "pmask"):
                m[f"{k}_{L}"] = np.ascontiguousarray(lay[c][k])
        in_maps.append(m)
    _last_in_maps = in_maps

    res = run_bass_kernel_spmd(nc, in_maps, core_ids=list(range(NCORES)))
    return np.float32(res.results[0]["out"][0, 0])
